# revision 9
# baseline (speedup 1.0000x reference)
"""ColorConstancy (multi-scale retinex) Trainium2 kernel.

Full-input contract: kernel(**inputs) takes the unsharded inputs from
setup_inputs() and returns the full (16, 3, 512, 512) float32 output.

Strategy (pure data parallel, batch sharded across 8 cores; 6 planes/core):
  log_img = ln(x + 1e-8)
  illum   = sum_s w_s * gauss2d_s(log_img)        (sigmas 2, 4, 8)
  refl    = log_img - illum
  out     = clip(exp((refl - mean) / (std_ddof1 + 1e-8)), 0, 1)

The 2-D Gaussian is separable: gauss2d_s(X) = U_s @ X @ U_s with U_s the
banded symmetric Toeplitz matrix of the 1-D kernel. Pass 1 computes
A_s = L^T V_s on the TensorEngine (banded: moving dim 128+2c cols per
matmul); the PSUM->SBUF evacuation negates (A_n = -A). Pass 2 accumulates
psi = I@L - sum_s A_n_s^T V_s = L - illum = refl directly in PSUM (an
identity matmul supplies +L), so no vector-engine subtract is needed; the
psi evacuation (plain copy, ACT or DVE) yields refl in fp16 SBUF. Stats
use DVE bn_stats/bn_aggr + a gpsimd partition_all_reduce (no PSUM bank),
tiny tail ops mostly on the otherwise-idle Pool engine, and the output is
one fused Exp activation (bias/scale = normalization) + min(.,1).

sigma=4,8 run pass 2 as fp8-e4m3 DoubleRow matmuls (2 contraction rows per
partition, 0.5 cyc/col); sigma=2 stays fp16 (precision: its center tap
dominates the response to extreme dark pixels). I/O is fp16 both ways.
PSUM: 2+2+2+2 banks, everything double-buffered.
"""

import numpy as np

N_CORES = 8
NPLANES = 6          # 2 batch images x 3 channels per core
H = W = 512
P = 128
NB = H // P          # 4 row blocks
CS = (6, 12, 24)     # band half-widths for sigma 2, 4, 8 (K = 13, 25, 49)
EPS = 1e-8
NPIX = H * W

_PROGRAM_CACHE = {}

# --- tuning knobs (per-mb engine patterns: 'A'=ACT, 'D'=DVE) ---
EV_A2 = "AAAA"       # pass-1 sigma2 evac engine per mb
EV_A4 = "AAAA"       # pass-1 sigma4 evac (ignored when MERGE_A48)
EV_A8 = "DDDD"       # pass-1 sigma8 evac (merged a48 evac when MERGE_A48)
EV_RF = "DDDD"       # pass-2 psi->refl evac
FP8_PASS2 = True     # sigma4/8 pass 2 as fp8e4 DoubleRow
MERGE_A48 = True     # sigma4+sigma8 share one 2-bank PSUM tile + one evac
USE_LAYERNORM = False  # gpsimd layernorm: F<=32 only, unusable for 2048
MIN_ON_POOL = True   # final min(yt,1) on Pool
TAIL_ON_POOL = True  # stats tail scalar ops on Pool
STATS_STRIDE = 1     # bn_stats column stride (2 = half-sample stats)
SBUF_BUFS = 2


def _ncol(kb, c):
    """Output column range that input row block kb touches through a band-c kernel."""
    return max(0, P * kb - c), min(W, P * (kb + 1) + c)


def build_program(reps=1, ablate=()):
    """Build + compile the per-core Bass program. reps>1 wraps the whole
    computation in a hardware loop (for timing by subtraction)."""
    ablate = set(ablate)
    import concourse.bacc as bacc
    import concourse.tile as tile
    from concourse import mybir, bass_isa

    f32 = mybir.dt.float32
    f16 = mybir.dt.float16
    f8 = mybir.dt.float8e4
    AF = mybir.ActivationFunctionType
    DR = mybir.MatmulPerfMode.DoubleRow

    # The activation-table chooser picks the first set containing each
    # function, which puts Ln in "natural_log" and Exp in "exp_and_others" and
    # reloads tables (~2.7us each) every plane. Narrow the cached table map so
    # only the combined "natural_log_exp_and_others" set provides Ln/Exp; then
    # one load serves the whole kernel.
    from concourse.hw_specs import get_activation_tables
    _tabs = get_activation_tables("gen3")
    for _name, _fset in _tabs.items():
        if _name != "natural_log_exp_and_others":
            _fset.discard(AF.Ln)
            _fset.discard(AF.Exp)

    nc = bacc.Bacc("TRN2", target_bir_lowering=False, debug=False,
                   num_devices=N_CORES)
    x = nc.declare_dram_parameter("x", [NPLANES, H, W], f16, isOutput=False)
    vs = [nc.declare_dram_parameter(f"v{s}", [H, W], f16, isOutput=False)
          for s in range(3)]
    vq = [nc.declare_dram_parameter(f"vq{s}", [H, W], f8, isOutput=False)
          for s in (1, 2)] if FP8_PASS2 else []
    ident = nc.declare_dram_parameter("ident", [P, P], f16, isOutput=False)
    y = nc.declare_dram_parameter("y", [NPLANES, H, W], f16, isOutput=True)

    with tile.TileContext(nc) as tc:
        with (
            tc.tile_pool(name="consts", bufs=1) as consts,
            tc.tile_pool(name="xin", bufs=SBUF_BUFS) as xpool,
            tc.tile_pool(name="logp", bufs=SBUF_BUFS) as lpool,
            tc.tile_pool(name="apool", bufs=SBUF_BUFS) as apool,
            tc.tile_pool(name="refl", bufs=SBUF_BUFS) as rpool,
            tc.tile_pool(name="yout", bufs=SBUF_BUFS) as ypool,
            tc.tile_pool(name="small", bufs=2) as spool,
            tc.tile_pool(name="ps2p", bufs=2, space="PSUM") as ps2p,
            tc.tile_pool(name="ps48p", bufs=2, space="PSUM") as ps48p,
            tc.tile_pool(name="psip", bufs=2, space="PSUM") as psip,
        ):
            # Banded blur matrices, resident for the whole kernel.
            # Layout [p, kb, n]: matrix row = kb*128 + p.
            V16 = []
            for s in range(3):
                vt = consts.tile([P, NB, W], f16, tag=f"v{s}")
                nc.sync.dma_start(
                    out=vt, in_=vs[s].rearrange("(kb p) n -> p kb n", p=P))
                V16.append(vt)
            V8Q = []
            for i, dram in enumerate(vq):
                vtq = consts.tile([P, NB, W], f8, tag=f"vq{i}")
                nc.sync.dma_start(
                    out=vtq, in_=dram.rearrange("(kb p) n -> p kb n", p=P))
                V8Q.append(vtq)
            i16 = consts.tile([P, P], f16, tag="ident")
            nc.sync.dma_start(out=i16, in_=ident[:, :])
            epst = consts.tile([P, 1], f32, tag="eps")
            nc.vector.memset(epst, EPS)

            tail_eng = nc.gpsimd if TAIL_ON_POOL else nc.vector

            def evac(eng, out, in_, negate):
                """PSUM -> SBUF evacuation on engine 'A'(CT) or 'D'(VE)."""
                if "evac" in ablate:
                    return
                if eng == "A":
                    nc.scalar.activation(out=out, in_=in_, func=AF.Copy,
                                         scale=-1.0 if negate else 1.0)
                else:
                    if negate:
                        nc.vector.tensor_scalar_mul(out=out, in0=in_,
                                                    scalar1=-1.0)
                    else:
                        nc.vector.tensor_copy(out=out, in_=in_)

            def emit_planes():
                state = {}

                def front(p):
                    # load -> ln(fp16) -> pass 1 (A_n_s = -L^T V_s)
                    xt = xpool.tile([P, NB, W], f16, tag="x")
                    nc.sync.dma_start(
                        out=xt, in_=x[p].rearrange("(kb q) w -> q kb w", q=P))
                    lt = lpool.tile([P, NB, W], f16, tag="l")
                    if "act" in ablate:
                        nc.scalar.copy(out=lt, in_=xt)
                    else:
                        nc.scalar.activation(out=lt, in_=xt, func=AF.Ln,
                                             bias=epst, scale=1.0)

                    adt = (f16, f8, f8) if FP8_PASS2 else (f16, f16, f16)
                    if MERGE_A48:
                        a2n = apool.tile([P, NB, W], f16, tag="a2")
                        a48n = apool.tile([P, 2, NB, W], f8, tag="a48")
                        An = (a2n, a48n)
                    else:
                        An = [apool.tile([P, NB, W], adt[s], tag=f"a{s}",
                                         name=f"a16_{s}")
                              for s in range(3)]
                    if "evac" in ablate:
                        for a in An:
                            nc.gpsimd.memset(a, 0.5)
                    for mb in range(NB):
                        ps2 = ps2p.tile([P, W], f32, tag="ps")
                        ps48 = ps48p.tile([P, 2, W], f32, tag="ps")
                        psv = (ps2, ps48[:, 0, :], ps48[:, 1, :])
                        if "pe" not in ablate:
                            for s in range(3):
                                for kb in range(NB):
                                    lo, hi = _ncol(kb, CS[s])
                                    nc.tensor.matmul(
                                        psv[s][:, lo:hi],
                                        lt[:, kb, P * mb:P * (mb + 1)],
                                        V16[s][:, kb, lo:hi],
                                        start=(kb == 0), stop=(kb == NB - 1),
                                    )
                        if MERGE_A48:
                            evac(EV_A2[mb], An[0][:, mb, :], ps2, negate=True)
                            evac(EV_A8[mb], An[1][:, :, mb, :], ps48,
                                 negate=True)
                        else:
                            evpat = (EV_A2, EV_A4, EV_A8)
                            for s in range(3):
                                evac(evpat[s][mb], An[s][:, mb, :], psv[s],
                                     negate=True)
                    state[p] = (lt, An)

                def back(p):
                    # pass 2: psi = L - illum in PSUM -> refl -> norm -> out
                    lt, An = state.pop(p)
                    if USE_LAYERNORM:
                        rt = rpool.tile([P, NB, W], f32, tag="r")
                    else:
                        rt = rpool.tile([P, NB, W], f16, tag="r")
                        st6 = spool.tile([P, NB, 6], f32, tag="st6")
                    for mb in range(NB):
                        psi = psip.tile([P, W], f32, tag="psi")
                        if "pe" not in ablate:
                            nc.tensor.matmul(
                                psi, i16, lt[:, mb, :],
                                start=True, stop=False)
                            # sigma2 fp16, kb-granular
                            for kb in range(NB):
                                lo, hi = _ncol(kb, CS[0])
                                nc.tensor.matmul(
                                    psi[:, lo:hi],
                                    An[0][:, kb, P * mb:P * (mb + 1)],
                                    V16[0][:, kb, lo:hi],
                                    start=False, stop=False,
                                )
                            if FP8_PASS2:
                                # sigma4/8 fp8 DoubleRow: 256-row kb-pairs
                                for si, s in enumerate((1, 2)):
                                    c = CS[s]
                                    if MERGE_A48:
                                        av = An[1][:, si]
                                    else:
                                        av = An[s]
                                    for kp in range(NB // 2):
                                        lo = max(0, 2 * P * kp - c)
                                        hi = min(W, 2 * P * (kp + 1) + c)
                                        nc.tensor.matmul(
                                            psi[:, lo:hi],
                                            av[:, 2 * kp:2 * kp + 2,
                                               P * mb:P * (mb + 1)],
                                            V8Q[si][:, 2 * kp:2 * kp + 2,
                                                    lo:hi],
                                            start=False,
                                            stop=(s == 2 and kp == 1),
                                            perf_mode=DR,
                                        )
                            else:
                                for s in (1, 2):
                                    for kb in range(NB):
                                        lo, hi = _ncol(kb, CS[s])
                                        nc.tensor.matmul(
                                            psi[:, lo:hi],
                                            An[s][:, kb, P * mb:P * (mb + 1)],
                                            V16[s][:, kb, lo:hi],
                                            start=False,
                                            stop=(s == 2 and kb == NB - 1),
                                        )
                        else:
                            nc.vector.memset(psi, 0.3)
                        evac(EV_RF[mb], rt[:, mb, :], psi, negate=False)
                        if not USE_LAYERNORM:
                            nc.vector.bn_stats(out=st6[:, mb, :],
                                               in_=rt[:, mb, ::STATS_STRIDE])

                    yt = ypool.tile([P, NB, W], f16, tag="y")
                    if USE_LAYERNORM:
                        # fused partition-axis layernorm over the whole plane
                        # (n_tokens=1 -> dmodel = 128*2048 = NPIX). ddof-0 vs
                        # ddof-1 and eps-in-sqrt differ by O(1e-6): negligible.
                        nt = rpool.tile([P, NB, W], f32, tag="norm")
                        nc.gpsimd.layernorm(
                            nt, rt, eps=EPS, subtract_mean=True, n_tokens=1)
                        if "act" in ablate:
                            nc.scalar.copy(out=yt, in_=nt)
                        else:
                            nc.scalar.activation(out=yt, in_=nt, func=AF.Exp)
                    else:
                        # plane-wide mean/var: per-partition bn stats, then a
                        # gpsimd all-reduce sums [mean_p, E[x^2]_p] across
                        # partitions (broadcast to all partitions, no PSUM).
                        mv = spool.tile([P, 2], f32, tag="mv")
                        nc.vector.bn_aggr(out=mv, in_=st6)
                        t2 = spool.tile([P, 2], f32, tag="t2")
                        tail_eng.tensor_mul(out=t2[:, 1:2], in0=mv[:, 0:1],
                                            in1=mv[:, 0:1])
                        tail_eng.tensor_add(out=t2[:, 1:2], in0=t2[:, 1:2],
                                            in1=mv[:, 1:2])
                        tail_eng.tensor_copy(out=t2[:, 0:1], in_=mv[:, 0:1])
                        ar = spool.tile([P, 2], f32, tag="ar")
                        nc.gpsimd.partition_all_reduce(
                            ar, t2, channels=P,
                            reduce_op=bass_isa.ReduceOp.add)

                        fin = spool.tile([P, 4], f32, tag="fin")
                        mean = fin[:, 0:1]
                        tmp = fin[:, 1:2]   # var -> std -> std+eps
                        rs = fin[:, 2:3]
                        nbv = fin[:, 3:4]
                        tail_eng.tensor_scalar_mul(out=mean, in0=ar[:, 0:1],
                                                   scalar1=1.0 / P)
                        sq = spool.tile([P, 1], f32, tag="sq")
                        tail_eng.tensor_mul(out=sq, in0=mean, in1=mean)
                        tail_eng.tensor_scalar_mul(out=tmp, in0=ar[:, 1:2],
                                                   scalar1=1.0 / P)
                        tail_eng.tensor_sub(out=tmp, in0=tmp, in1=sq)
                        # std = exp(0.5*ln(var*N/(N-1))) (ddof=1), avoiding
                        # the sqrt table set; Ln/Exp share one ACT table set.
                        npix = NPIX // STATS_STRIDE
                        nc.scalar.activation(out=tmp, in_=tmp, func=AF.Ln,
                                             scale=float(npix) / (npix - 1))
                        nc.scalar.activation(out=tmp, in_=tmp, func=AF.Exp,
                                             scale=0.5)
                        tail_eng.tensor_scalar_add(out=tmp, in0=tmp,
                                                   scalar1=EPS)
                        nc.vector.reciprocal(out=rs, in_=tmp)
                        # nbv = -mean * rs (one fused tensor_scalar: two ops)
                        nc.vector.tensor_scalar(out=nbv, in0=mean, scalar1=rs,
                                                scalar2=-1.0,
                                                op0=mybir.AluOpType.mult,
                                                op1=mybir.AluOpType.mult)
                        if "act" in ablate:
                            nc.scalar.copy(out=yt, in_=rt)
                        else:
                            nc.scalar.activation(out=yt, in_=rt, func=AF.Exp,
                                                 bias=nbv, scale=rs)
                    if MIN_ON_POOL:
                        nc.gpsimd.tensor_scalar_min(out=yt, in0=yt,
                                                    scalar1=1.0)
                    else:
                        nc.vector.tensor_scalar_min(out=yt, in0=yt,
                                                    scalar1=1.0)
                    nc.sync.dma_start(
                        out=y[p].rearrange("(kb q) w -> q kb w", q=P), in_=yt)

                # software-pipelined: pass 1 of plane p overlaps pass 2 of p-1
                for p in range(NPLANES + 1):
                    if p < NPLANES:
                        front(p)
                    if p >= 1:
                        back(p - 1)

            if isinstance(reps, str) and reps.startswith("u"):
                for _ in range(int(reps[1:])):
                    emit_planes()
            elif reps == 1:
                emit_planes()
            else:
                from concourse import mybir as _mb
                with tc.For_i(0, reps, 1,
                              hint_engines=(_mb.EngineType.PE,)):
                    emit_planes()

    nc.compile()
    return nc


def get_program(reps=1):
    if reps not in _PROGRAM_CACHE:
        _PROGRAM_CACHE[reps] = build_program(reps)
    return _PROGRAM_CACHE[reps]


def _u_factors(k0, k1, k2):
    w = np.array([1.0, 0.75, 0.5], dtype=np.float64)
    w /= w.sum()
    us = []
    for s, k2d in enumerate((k0, k1, k2)):
        g = np.asarray(k2d)[0, 0].astype(np.float64)
        us.append((g.sum(axis=0), np.sqrt(w[s])))
    return us


def _toeplitz(u, scale, dtype, feedback=False):
    """Banded symmetric Toeplitz of scale*u. With feedback=True, quantize
    with per-column running error compensation (kills net kernel-mass bias
    from coarse dtypes like fp8)."""
    c = len(u) // 2
    V64 = np.zeros((H, W), dtype=np.float64)
    for d in range(-c, c + 1):
        V64 += np.diag(np.full(H - abs(d), scale * u[c + d]), k=d)
    if not feedback:
        return V64.astype(dtype)
    Vq = np.zeros((H, W), dtype=dtype)
    for j in range(W):
        lo, hi = max(0, j - c), min(H, j + c + 1)
        carry = 0.0
        for i in range(lo, hi):
            q = np.asarray(V64[i, j] + carry, dtype=dtype)
            carry = (V64[i, j] + carry) - float(q)
            Vq[i, j] = q
    return Vq


def build_v_matrices(k0, k1, k2):
    """fp16 banded Toeplitz matrices sqrt(w_s) * toeplitz(u_s) from the
    reference's 2-D depthwise kernels (u_s = column sums of the normalized
    2-D kernel, exact by separability)."""
    return [_toeplitz(u, sc, np.float16) for u, sc in _u_factors(k0, k1, k2)]


def build_const_inputs(k0, k1, k2):
    """Per-core constant input tensors (same on every core)."""
    us = _u_factors(k0, k1, k2)
    m = {f"v{s}": _toeplitz(u, sc, np.float16)
         for s, (u, sc) in enumerate(us)}
    if FP8_PASS2:
        import ml_dtypes
        for i, s in enumerate((1, 2)):
            u, sc = us[s]
            m[f"vq{i}"] = _toeplitz(u, sc, ml_dtypes.float8_e4m3,
                                    feedback=True)
    m["ident"] = np.eye(P, dtype=np.float16)
    return m


def kernel(rgb_image, k0, k1, k2):
    from concourse.bass_utils import run_bass_kernel_spmd

    nc = get_program()
    consts = build_const_inputs(k0, k1, k2)
    xs = np.asarray(rgb_image, dtype=np.float16)
    B = xs.shape[0]
    per_core = B // N_CORES
    in_maps = []
    for c in range(N_CORES):
        m = {"x": np.ascontiguousarray(
            xs[c * per_core:(c + 1) * per_core].reshape(NPLANES, H, W))}
        m.update(consts)
        in_maps.append(m)
    res = run_bass_kernel_spmd(nc, in_maps, list(range(N_CORES)))
    out = np.empty((B, 3, H, W), dtype=np.float32)
    for c in range(N_CORES):
        out[c * per_core:(c + 1) * per_core] = (
            res.results[c]["y"].astype(np.float32).reshape(per_core, 3, H, W))
    return out
